# revision 6
# baseline (speedup 1.0000x reference)
"""Trainium2 kernel for the cross-attention + fusion + pooled-FFN model.

Pure data parallel over the batch axis across the 8 NeuronCores
(512 items per core, weights replicated, no cross-item communication
until the final FFN/BN which is computed per-shard).

Note: this environment's walrus build rejects any compute instruction
carrying two embedded semaphore waits ("Too many sync wait commands",
CoreV2GenImpl.cpp:176 — reproduced with a minimal 1-matmul Tile
kernel), which blocks the hand-written Bass/Tile path entirely.  The
kernel therefore lowers through PJRT/XLA-Neuron instead: one jit per
device shard, heavy matmuls in bf16 with fp32 accumulation (full PE
array rate), softmax/pooling/BatchNorm in fp32.

Self-contained: hardcodes all shapes; no sibling imports.
"""

import numpy as np
import jax
import jax.numpy as jnp

B, N, D, P = 4096, 32, 768, 512
NCORES = 8
BL = B // NCORES
BN_EPS = 1e-5
F32 = jnp.float32
BF16 = jnp.bfloat16


def _shard_fn(content, image, Wq, bq, Wk, bk, Wv, bv, W1, b1, W2, b2,
              bn_gamma, bn_beta, bn_mean, bn_var):
    cb, ib = content.astype(BF16), image.astype(BF16)
    q = jnp.einsum("bnd,dp->bnp", cb, Wq.astype(BF16),
                   preferred_element_type=F32) + bq
    k = jnp.einsum("bmd,dp->bmp", ib, Wk.astype(BF16),
                   preferred_element_type=F32) + bk
    v = jnp.einsum("bmd,dp->bmp", ib, Wv.astype(BF16),
                   preferred_element_type=F32) + bv
    scores = jnp.einsum("bnp,bmp->bnm", q.astype(BF16), k.astype(BF16),
                        preferred_element_type=F32) / jnp.sqrt(F32(P))
    attn = jax.nn.softmax(scores, axis=-1)
    align = jnp.einsum("bnm,bmp->bnp", attn.astype(BF16), v.astype(BF16),
                       preferred_element_type=F32)
    sub = q - align
    dot = jnp.sum(q * align, axis=-1, keepdims=True)
    final = jnp.concatenate([q, align, sub, dot], axis=-1)
    pooled = jnp.concatenate([final.mean(axis=1), final.max(axis=1)], axis=-1)
    h = jax.nn.relu(jnp.einsum("bf,fd->bd", pooled.astype(BF16),
                               W1.astype(BF16), preferred_element_type=F32) + b1)
    y = jnp.einsum("bd,do->bo", h.astype(BF16), W2.astype(BF16),
                   preferred_element_type=F32) + b2
    y = (y - bn_mean) * jax.lax.rsqrt(bn_var + BN_EPS) * bn_gamma + bn_beta
    return y


_JIT = {}


def kernel(**inputs) -> np.ndarray:
    devs = jax.devices()[:NCORES]
    content = np.asarray(inputs["content_res"], np.float32)
    image = np.asarray(inputs["image_res"], np.float32)
    weight_names = ["Wq", "bq", "Wk", "bk", "Wv", "bv", "W1", "b1", "W2", "b2",
                    "bn_gamma", "bn_beta", "bn_mean", "bn_var"]
    weights = [np.asarray(inputs[w], np.float32) for w in weight_names]

    if "fn" not in _JIT:
        _JIT["fn"] = jax.jit(_shard_fn)
    fn = _JIT["fn"]

    # async dispatch: all 8 device shards run concurrently
    outs = []
    for r, d in enumerate(devs):
        sl = slice(r * BL, (r + 1) * BL)
        args = [jax.device_put(content[sl], d), jax.device_put(image[sl], d)]
        args += [jax.device_put(w, d) for w in weights]
        outs.append(fn(*args))
    y = np.concatenate([np.asarray(o) for o in outs], axis=0)
    return y.astype(np.float32)


# revision 7
# speedup vs baseline: 1.7981x; 1.7981x over previous
"""Trainium2 kernel for the cross-attention + fusion + pooled-FFN model.

Pure data parallel over the batch axis across the 8 NeuronCores
(512 items per core, weights replicated, no cross-item communication
until the final FFN/BN which is computed per-shard).

Note: this environment's walrus build rejects any compute instruction
carrying two embedded semaphore waits ("Too many sync wait commands",
CoreV2GenImpl.cpp:176 — reproduced with a minimal 1-matmul Tile
kernel), which blocks the hand-written Bass/Tile path entirely.  The
kernel therefore lowers through PJRT/XLA-Neuron instead: one jit per
device shard, heavy matmuls in bf16 with fp32 accumulation (full PE
array rate), softmax/pooling/BatchNorm in fp32.

Self-contained: hardcodes all shapes; no sibling imports.
"""

import ml_dtypes
import numpy as np
import jax
import jax.numpy as jnp

B, N, D, P = 4096, 32, 768, 512
NCORES = 8
BL = B // NCORES
BN_EPS = 1e-5
F32 = jnp.float32
BF16 = jnp.bfloat16


def _shard_fn(content, image, Wq, bq, Wk, bk, Wv, bv, W1, b1, W2, b2,
              bn_gamma, bn_beta, bn_mean, bn_var):
    cb, ib = content.astype(BF16), image.astype(BF16)
    q = jnp.einsum("bnd,dp->bnp", cb, Wq.astype(BF16),
                   preferred_element_type=F32) + bq
    k = jnp.einsum("bmd,dp->bmp", ib, Wk.astype(BF16),
                   preferred_element_type=F32) + bk
    v = jnp.einsum("bmd,dp->bmp", ib, Wv.astype(BF16),
                   preferred_element_type=F32) + bv
    scores = jnp.einsum("bnp,bmp->bnm", q.astype(BF16), k.astype(BF16),
                        preferred_element_type=F32) / jnp.sqrt(F32(P))
    attn = jax.nn.softmax(scores, axis=-1)
    align = jnp.einsum("bnm,bmp->bnp", attn.astype(BF16), v.astype(BF16),
                       preferred_element_type=F32)
    sub = q - align
    dot = jnp.sum(q * align, axis=-1, keepdims=True)
    final = jnp.concatenate([q, align, sub, dot], axis=-1)
    pooled = jnp.concatenate([final.mean(axis=1), final.max(axis=1)], axis=-1)
    h = jax.nn.relu(jnp.einsum("bf,fd->bd", pooled.astype(BF16),
                               W1.astype(BF16), preferred_element_type=F32) + b1)
    y = jnp.einsum("bd,do->bo", h.astype(BF16), W2.astype(BF16),
                   preferred_element_type=F32) + b2
    y = (y - bn_mean) * jax.lax.rsqrt(bn_var + BN_EPS) * bn_gamma + bn_beta
    return y


_JIT = {}


def kernel(**inputs) -> np.ndarray:
    devs = jax.devices()[:NCORES]
    content = np.asarray(inputs["content_res"], np.float32)
    image = np.asarray(inputs["image_res"], np.float32)
    weight_names = ["Wq", "bq", "Wk", "bk", "Wv", "bv", "W1", "b1", "W2", "b2",
                    "bn_gamma", "bn_beta", "bn_mean", "bn_var"]
    weights = [np.asarray(inputs[w], np.float32) for w in weight_names]

    if "fn" not in _JIT:
        _JIT["fn"] = jax.jit(_shard_fn)
    fn = _JIT["fn"]

    # async dispatch: all 8 device shards run concurrently
    outs = []
    for r, d in enumerate(devs):
        sl = slice(r * BL, (r + 1) * BL)
        args = [
            jax.device_put(content[sl].astype(ml_dtypes.bfloat16), d),
            jax.device_put(image[sl].astype(ml_dtypes.bfloat16), d),
        ]
        args += [jax.device_put(w, d) for w in weights]
        outs.append(fn(*args))
    y = np.concatenate([np.asarray(o) for o in outs], axis=0)
    return y.astype(np.float32)
